# revision 8
# baseline (speedup 1.0000x reference)
"""Trainium2 Bass kernel for nn_ActorNetwork (GNN boundary-node actor head).

Strategy (data-parallel over node ranges, no collectives):
- Host packs a per-core bf16 table [125000, 256]: cols 0:128 node embedding,
  128:192 additive action mask (0 / -1e9), 192 batch index. Boundary indices
  are bucketed by owning core and by int16-addressable sub-table, then
  remapped to sub-table-local int16.
- Each core runs dma_gather(transpose=True) over its boundary rows: gathered
  rows land feature-major [128, 2, n] in SBUF (embeddings pre-transposed for
  the TensorEngine, mask and batch-idx rows inline).
- The scatter-mean + global-encoder MLP is computed on-device per core
  (replicated, it is tiny), folded into layer-1 tables G1 = gc @ W1[128:] + b1.
- Per 512-row tile: one-hot(batch) via a K=1 broadcast matmul + is_equal,
  then bf16 matmuls for both 3-layer MLPs; mask applied as an add; the
  [rows, 64] partition-logits output is PE-transposed back to row-major.
- Host scatters per-core outputs back into the original boundary order.
"""

import sys

sys.path.insert(0, "/opt/trn_rl_repo")

import ml_dtypes
import numpy as np

import concourse.bacc as bacc
import concourse.bass as bass
import concourse.mybir as mybir
import concourse.tile as tile
from concourse.bass_utils import run_bass_kernel_spmd

F32 = mybir.dt.float32
BF16 = mybir.dt.bfloat16
I16 = mybir.dt.int16
I32 = mybir.dt.int32
RELU = mybir.ActivationFunctionType.Relu
ADD = mybir.AluOpType.add
MULT = mybir.AluOpType.mult
ISEQ = mybir.AluOpType.is_equal

CORES = 8
N_NODES = 1_000_000
NPC = N_NODES // CORES          # nodes per core
SUB = 4                         # sub-tables per core (int16 index range)
SUB_ROWS = NPC // SUB           # 31250 <= 32767
ELEM = 256                      # bf16 elements per packed row (512B)
D = 128                         # node/region embedding dim
B = 64                          # graphs per batch
NREG = 2048                     # regions
H = 256
H2 = 128
PQ = 64                         # partitions (action dim)
TILE = 512                      # rows per compute tile
CHUNK = 512                    # rows per dma_gather
MASK_NEG = -1e9


def _build(nchunk: int, reps: int = 1):
    """Build + compile the SPMD graph for NR = nchunk*CHUNK boundary rows/core.
    reps>1 wraps the whole body in a For_i loop (timing builds only)."""
    nr = nchunk * CHUNK
    nsc = nchunk // SUB  # chunks per sub-table

    nc = bacc.Bacc("TRN2", target_bir_lowering=False, debug=False, num_devices=CORES)

    table_d = nc.dram_tensor("table", [NPC, ELEM], BF16, kind="ExternalInput")
    idxs_d = nc.dram_tensor("idxs", [nchunk, 128, CHUNK // 16], I16, kind="ExternalInput")
    rbit_d = nc.dram_tensor("rbi_t", [128, NREG // 128], F32, kind="ExternalInput")
    rege_d = nc.dram_tensor("reg_emb", [NREG, D], F32, kind="ExternalInput")
    gew1_d = nc.dram_tensor("ge_w1", [D, H], F32, kind="ExternalInput")
    geb1_d = nc.dram_tensor("ge_b1t", [128, H // 128], F32, kind="ExternalInput")
    gew2_d = nc.dram_tensor("ge_w2", [H, D], F32, kind="ExternalInput")
    geb2_d = nc.dram_tensor("ge_b2t", [128, 1], F32, kind="ExternalInput")
    w1a_d, w1b_d, b1r_d, w2_d, b2t_d, w3_d = {}, {}, {}, {}, {}, {}
    for m in ("ns", "ps"):
        w1a_d[m] = nc.dram_tensor(f"{m}_w1a", [D, H], BF16, kind="ExternalInput")
        w1b_d[m] = nc.dram_tensor(f"{m}_w1b", [D, H], F32, kind="ExternalInput")
        b1r_d[m] = nc.dram_tensor(f"{m}_b1r", [1, H], F32, kind="ExternalInput")
        w2_d[m] = nc.dram_tensor(f"{m}_w2", [H, H2], BF16, kind="ExternalInput")
        b2t_d[m] = nc.dram_tensor(f"{m}_b2t", [128, 1], F32, kind="ExternalInput")
    w3_d["ns"] = nc.dram_tensor("ns_w3", [H2, 1], BF16, kind="ExternalInput")
    nsb3_d = nc.dram_tensor("ns_b3r", [1, 1], F32, kind="ExternalInput")
    w3_d["ps"] = nc.dram_tensor("ps_w3", [H2, PQ], BF16, kind="ExternalInput")
    psb3_d = nc.dram_tensor("ps_b3r", [1, PQ], BF16, kind="ExternalInput")

    idn_d = nc.dram_tensor("idn", [128, 128], F32, kind="ExternalInput")
    iotab_d = nc.dram_tensor("iotab", [B, 1], F32, kind="ExternalInput")
    iotar_d = nc.dram_tensor("iotar", [128, B], F32, kind="ExternalInput")

    nsout_d = nc.dram_tensor("ns_out", [nr], F32, kind="ExternalOutput")
    psout_d = nc.dram_tensor("ps_out", [nr, PQ], F32, kind="ExternalOutput")

    with tile.TileContext(nc) as tc:
        with (
            tc.tile_pool(name="sb", bufs=1) as sb,
            tc.tile_pool(name="pp", bufs=1, space="PSUM") as pp,
        ):
            def body():
                # ---------- constants (host-uploaded; keep gpsimd to dma_gather only) ----------
                idn = sb.tile([128, 128], F32, tag="idn")
                nc.sync.dma_start(idn[:], idn_d.ap())
                iota_bf = sb.tile([B, 1], F32, tag="iotbf")
                nc.sync.dma_start(iota_bf[:], iotab_d.ap())
                iota_rf = sb.tile([128, B], F32, tag="iotrf")
                nc.sync.dma_start(iota_rf[:], iotar_d.ap())
                ones_col = sb.tile([128, 1], F32, tag="onec")
                nc.vector.memset(ones_col[:], 1.0)
                ones1f = sb.tile([1, B], F32, tag="one1f")
                nc.vector.memset(ones1f[:], 1.0)
                ones1r = sb.tile([1, TILE], BF16, tag="one1r")
                nc.vector.memset(ones1r[:], 1.0)

                # ---------- small-weight loads ----------
                rbi_sb = sb.tile([128, NREG // 128], F32, tag="rbi")
                nc.sync.dma_start(rbi_sb[:], rbit_d.ap())
                reg_sb = sb.tile([128, NREG // 128, D], F32, tag="rege")
                nc.sync.dma_start(
                    reg_sb[:], rege_d.ap().rearrange("(k p) d -> p k d", p=128)
                )
                gew1_sb = sb.tile([D, H], F32, tag="gew1")
                nc.sync.dma_start(gew1_sb[:], gew1_d.ap())
                geb1_sb = sb.tile([128, H // 128], F32, tag="geb1")
                nc.sync.dma_start(geb1_sb[:], geb1_d.ap())
                gew2_sb = sb.tile([128, H // 128, D], F32, tag="gew2")
                nc.sync.dma_start(
                    gew2_sb[:], gew2_d.ap().rearrange("(c p) d -> p c d", p=128)
                )
                geb2_sb = sb.tile([128, 1], F32, tag="geb2")
                nc.sync.dma_start(geb2_sb[:], geb2_d.ap())
                w1a_sb, w1b_sb, b1r_sb, w2_sb, b2t_sb, w3_sb = {}, {}, {}, {}, {}, {}
                for m in ("ns", "ps"):
                    w1a_sb[m] = sb.tile([D, H], BF16, tag=f"w1a{m}", name=f"w1a{m}")
                    nc.sync.dma_start(w1a_sb[m][:], w1a_d[m].ap())
                    w1b_sb[m] = sb.tile([D, H], F32, tag=f"w1b{m}", name=f"w1b{m}")
                    nc.sync.dma_start(w1b_sb[m][:], w1b_d[m].ap())
                    b1r_sb[m] = sb.tile([1, H], F32, tag=f"b1r{m}", name=f"b1r{m}")
                    nc.sync.dma_start(b1r_sb[m][:], b1r_d[m].ap())
                    w2_sb[m] = sb.tile([128, H // 128, H2], BF16, tag=f"w2{m}", name=f"w2{m}")
                    nc.sync.dma_start(
                        w2_sb[m][:], w2_d[m].ap().rearrange("(c p) d -> p c d", p=128)
                    )
                    b2t_sb[m] = sb.tile([128, 1], F32, tag=f"b2t{m}", name=f"b2t{m}")
                    nc.sync.dma_start(b2t_sb[m][:], b2t_d[m].ap())
                w3_sb["ns"] = sb.tile([H2, 1], BF16, tag="w3ns", name="w3ns")
                nc.sync.dma_start(w3_sb["ns"][:], w3_d["ns"].ap())
                w3_sb["ps"] = sb.tile([H2, PQ], BF16, tag="w3ps", name="w3ps")
                nc.sync.dma_start(w3_sb["ps"][:], w3_d["ps"].ap())
                nsb3_sb = sb.tile([1, 1], F32, tag="nsb3")
                nc.sync.dma_start(nsb3_sb[:], nsb3_d.ap())
                psb3_sb = sb.tile([1, PQ], BF16, tag="psb3")
                nc.sync.dma_start(psb3_sb[:], psb3_d.ap())

                # ---------- global context (scatter-mean + encoder MLP) ----------
                sums_ps = pp.tile([B, D], F32, tag="preA")
                cnts_ps = pp.tile([B, 1], F32, tag="preB")
                nk = NREG // 128
                for k in range(nk):
                    ohk = sb.tile([128, B], F32, tag="ohr", bufs=2)
                    nc.vector.tensor_tensor(
                        out=ohk[:],
                        in0=rbi_sb[:, k : k + 1].to_broadcast([128, B]),
                        in1=iota_rf[:],
                        op=ISEQ,
                    )
                    nc.tensor.matmul(
                        sums_ps[:], lhsT=ohk[:], rhs=reg_sb[:, k, :],
                        start=(k == 0), stop=(k == nk - 1),
                    )
                    nc.tensor.matmul(
                        cnts_ps[:], lhsT=ohk[:], rhs=ones_col[:],
                        start=(k == 0), stop=(k == nk - 1),
                    )
                cmax = sb.tile([B, 1], F32, tag="cmax")
                nc.vector.tensor_scalar_max(cmax[:], cnts_ps[:], 1.0)
                crcp = sb.tile([B, 1], F32, tag="crcp")
                nc.vector.reciprocal(crcp[:], cmax[:])
                rm_sb = sb.tile([B, D], F32, tag="rm")
                nc.vector.tensor_tensor(
                    out=rm_sb[:], in0=sums_ps[:],
                    in1=crcp[:].to_broadcast([B, D]), op=MULT,
                )
                rmT_ps = pp.tile([D, B], F32, tag="preA")
                nc.tensor.transpose(rmT_ps[:], rm_sb[:], idn[:B, :B])
                rmT_sb = sb.tile([D, B], F32, tag="rmT")
                nc.vector.tensor_copy(rmT_sb[:], rmT_ps[:])
                geh_sb = []
                for c in range(H // 128):
                    hps = pp.tile([128, B], F32, tag="preA")
                    nc.tensor.matmul(
                        hps[:], lhsT=gew1_sb[:, c * 128 : (c + 1) * 128],
                        rhs=rmT_sb[:], start=True, stop=True,
                    )
                    hsb = sb.tile([128, B], F32, tag="geh", bufs=2)
                    nc.scalar.activation(hsb[:], hps[:], RELU, bias=geb1_sb[:, c : c + 1])
                    geh_sb.append(hsb)
                gc_ps = pp.tile([D, B], F32, tag="preA")
                for c in range(H // 128):
                    nc.tensor.matmul(
                        gc_ps[:], lhsT=gew2_sb[:, c, :], rhs=geh_sb[c][:],
                        start=(c == 0), stop=(c == H // 128 - 1),
                    )
                gcT_sb = sb.tile([D, B], F32, tag="gcT")
                nc.vector.tensor_scalar(
                    gcT_sb[:], gc_ps[:], geb2_sb[:, :1], None, op0=ADD
                )
                g1_sb = {}
                for m in ("ns", "ps"):
                    g1ps = pp.tile([B, H], F32, tag="preA")
                    nc.tensor.matmul(
                        g1ps[:], lhsT=gcT_sb[:], rhs=w1b_sb[m][:],
                        start=True, stop=False,
                    )
                    nc.tensor.matmul(
                        g1ps[:], lhsT=ones1f[:], rhs=b1r_sb[m][:],
                        start=False, stop=True,
                    )
                    g1_sb[m] = sb.tile([B, H], BF16, tag=f"g1{m}", name=f"g1{m}")
                    nc.vector.tensor_copy(g1_sb[m][:], g1ps[:])

                # ---------- main loop over gather chunks ----------
                nsout_v = nsout_d.ap().rearrange("(k f) -> k f", k=nchunk)
                psout_v = psout_d.ap().rearrange(
                    "(k q p) d -> k p q d", p=128, q=CHUNK // 128
                )
                for k in range(nchunk):
                    s = k // nsc
                    idxt = sb.tile([128, CHUNK // 16], I16, tag="idx", bufs=3)
                    nc.sync.dma_start(idxt[:], idxs_d.ap()[k])
                    g = sb.tile([128, 2, CHUNK], BF16, tag="g", bufs=2)
                    nc.gpsimd.dma_gather(
                        g[:],
                        table_d.ap()[s * SUB_ROWS : (s + 1) * SUB_ROWS, :],
                        idxt[:],
                        CHUNK,
                        CHUNK,
                        ELEM,
                        transpose=True,
                    )
                    msk0 = sb.tile([PQ, CHUNK], BF16, tag="msk0", bufs=2)
                    nc.sync.dma_start(msk0[:], g[PQ:128, 1, :])
                    nso = sb.tile([1, CHUNK], F32, tag="nso", bufs=2)
                    pso = sb.tile([128, CHUNK // 128, PQ], F32, tag="pso", bufs=2)
                    for t in range(CHUNK // TILE):
                        sl = slice(t * TILE, (t + 1) * TILE)
                        emb = g[:, 0, sl]
                        msk = msk0[:, sl]
                        bb64 = g[0:B, 1, sl]  # batch idx, replicated on 64 partitions
                        oh = sb.tile([B, TILE], BF16, tag="oh", bufs=2)
                        nc.vector.tensor_tensor(
                            out=oh[:], in0=bb64,
                            in1=iota_bf[:].to_broadcast([B, TILE]), op=ISEQ,
                        )
                        for m in ("ns", "ps"):
                            h1 = []
                            for c in range(H // 128):
                                l1 = pp.tile([128, TILE], F32, tag="big", bufs=3)
                                nc.tensor.matmul(
                                    l1[:], lhsT=w1a_sb[m][:, c * 128 : (c + 1) * 128],
                                    rhs=emb, start=True, stop=False,
                                )
                                nc.tensor.matmul(
                                    l1[:], lhsT=g1_sb[m][:, c * 128 : (c + 1) * 128],
                                    rhs=oh[:], start=False, stop=True,
                                )
                                hc = sb.tile([128, TILE], BF16, tag="h1", bufs=4)
                                if c == 0:
                                    nc.scalar.activation(hc[:], l1[:], RELU)
                                else:
                                    nc.vector.tensor_scalar_max(hc[:], l1[:], 0.0)
                                h1.append(hc)
                            l2 = pp.tile([128, TILE], F32, tag="big", bufs=3)
                            for c in range(H // 128):
                                nc.tensor.matmul(
                                    l2[:], lhsT=w2_sb[m][:, c, :], rhs=h1[c][:],
                                    start=(c == 0), stop=(c == H // 128 - 1),
                                )
                            h2 = sb.tile([H2, TILE], BF16, tag="h2", bufs=3)
                            nc.scalar.activation(h2[:], l2[:], RELU, bias=b2t_sb[m][:, :1])
                            if m == "ns":
                                l3 = pp.tile([1, TILE], F32, tag="small", bufs=3)
                                nc.tensor.matmul(
                                    l3[:], lhsT=w3_sb["ns"][:], rhs=h2[:],
                                    start=True, stop=True,
                                )
                                nc.vector.tensor_scalar(
                                    nso[:, sl], l3[:], nsb3_sb[:1, :1], None, op0=ADD
                                )
                            else:
                                l3p = pp.tile([PQ, TILE], F32, tag="small", bufs=3)
                                nc.tensor.matmul(
                                    l3p[:], lhsT=w3_sb["ps"][:], rhs=h2[:],
                                    start=True, stop=False,
                                )
                                nc.tensor.matmul(
                                    l3p[:], lhsT=psb3_sb[:], rhs=ones1r[:],
                                    start=False, stop=True,
                                )
                                mskd = sb.tile([PQ, TILE], F32, tag="mskd", bufs=2)
                                nc.vector.tensor_tensor(
                                    out=mskd[:], in0=l3p[:], in1=msk, op=ADD
                                )
                                tp = pp.tile([128, (TILE // 128) * PQ], F32, tag="small", bufs=3)
                                for j in range(TILE // 128):
                                    nc.tensor.transpose(
                                        tp[:, j * PQ : (j + 1) * PQ],
                                        mskd[:, j * 128 : (j + 1) * 128],
                                        idn[:PQ, :PQ],
                                    )
                                nc.scalar.activation(
                                    pso[:, t * (TILE // 128) : (t + 1) * (TILE // 128), :]
                                    .rearrange("p a b -> p (a b)"),
                                    tp[:],
                                    mybir.ActivationFunctionType.Copy,
                                )
                    nc.sync.dma_start(nsout_v[k : k + 1, :], nso[:])
                    nc.sync.dma_start(psout_v[k], pso[:])

            if reps == 1:
                body()
            else:
                with tc.For_i(0, reps, 1):
                    body()
    nc.compile()
    return nc


def _prep(inputs):
    """Host-side shard/remap/pack. Returns (in_maps, glob_pos, nchunk, nb)."""
    f32 = np.float32
    bn = np.asarray(inputs["boundary_nodes"]).astype(np.int64).ravel()
    nbi = np.asarray(inputs["node_batch_idx"]).astype(np.int64).ravel()
    rbi = np.asarray(inputs["region_batch_idx"]).astype(np.int64).ravel()
    am = np.asarray(inputs["action_mask"])
    emb = np.asarray(inputs["node_embeddings"], dtype=f32)
    rege = np.asarray(inputs["region_embeddings"], dtype=f32)
    nb = bn.shape[0]

    # packed per-node table
    pk = np.zeros((N_NODES, ELEM), dtype=ml_dtypes.bfloat16)
    pk[:, :D] = emb
    pk[:, D : D + B] = nbi.astype(np.float32)[:, None]
    pk[:, D + B :] = np.where(am, 0.0, MASK_NEG).astype(f32)

    core = bn // NPC
    local = bn - core * NPC
    sub = local // SUB_ROWS

    pos = [
        [np.flatnonzero((core == c) & (sub == s)) for s in range(SUB)]
        for c in range(CORES)
    ]
    maxcnt = max(len(p) for row in pos for p in row)
    ns_rows = max(CHUNK, ((maxcnt + CHUNK - 1) // CHUNK) * CHUNK)
    nr = SUB * ns_rows
    nchunk = nr // CHUNK

    in_maps = []
    glob_pos = []
    for c in range(CORES):
        idx16 = np.zeros(nr, dtype=np.int16)
        gp = np.full(nr, -1, dtype=np.int64)
        for s in range(SUB):
            p = pos[c][s]
            v = (local[p] - s * SUB_ROWS).astype(np.int16)
            idx16[s * ns_rows : s * ns_rows + len(v)] = v
            gp[s * ns_rows : s * ns_rows + len(p)] = p
        # dma_gather index layout: index i at [i % 16, i // 16], replicated x8
        arr = idx16.reshape(nchunk, CHUNK // 16, 16).transpose(0, 2, 1)
        idxs = np.tile(arr, (1, 8, 1))
        glob_pos.append(gp)

        m = {
            "idn": np.eye(128, dtype=f32),
            "iotab": np.arange(B, dtype=f32).reshape(B, 1),
            "iotar": np.tile(np.arange(B, dtype=f32), (128, 1)),
            "table": pk[c * NPC : (c + 1) * NPC],
            "idxs": np.ascontiguousarray(idxs),
            "rbi_t": np.ascontiguousarray(
                rbi.astype(f32).reshape(NREG // 128, 128).T
            ),
            "reg_emb": rege,
            "ge_w1": np.asarray(inputs["ge_w1"], dtype=f32),
            "ge_b1t": np.ascontiguousarray(
                np.asarray(inputs["ge_b1"], dtype=f32).reshape(H // 128, 128).T
            ),
            "ge_w2": np.asarray(inputs["ge_w2"], dtype=f32),
            "ge_b2t": np.asarray(inputs["ge_b2"], dtype=f32).reshape(128, 1),
        }
        for mm in ("ns", "ps"):
            w1 = np.asarray(inputs[f"{mm}_w1"], dtype=f32)
            m[f"{mm}_w1a"] = w1[:D].astype(ml_dtypes.bfloat16)
            m[f"{mm}_w1b"] = np.ascontiguousarray(w1[D:])
            m[f"{mm}_b1r"] = np.asarray(inputs[f"{mm}_b1"], dtype=f32).reshape(1, H)
            m[f"{mm}_w2"] = np.asarray(inputs[f"{mm}_w2"], dtype=f32).astype(
                ml_dtypes.bfloat16
            )
            m[f"{mm}_b2t"] = np.asarray(inputs[f"{mm}_b2"], dtype=f32).reshape(128, 1)
        m["ns_w3"] = np.asarray(inputs["ns_w3"], dtype=f32).astype(ml_dtypes.bfloat16)
        m["ns_b3r"] = np.asarray(inputs["ns_b3"], dtype=f32).reshape(1, 1)
        m["ps_w3"] = np.asarray(inputs["ps_w3"], dtype=f32).astype(ml_dtypes.bfloat16)
        m["ps_b3r"] = (
            np.asarray(inputs["ps_b3"], dtype=f32)
            .astype(ml_dtypes.bfloat16)
            .reshape(1, PQ)
        )
        in_maps.append(m)
    return in_maps, glob_pos, nchunk, nb


_BUILD_CACHE: dict = {}


def kernel(**inputs):
    in_maps, glob_pos, nchunk, nb = _prep(inputs)
    nc = _BUILD_CACHE.get(nchunk)
    if nc is None:
        nc = _build(nchunk)
        _BUILD_CACHE[nchunk] = nc
    res = run_bass_kernel_spmd(nc, in_maps, core_ids=list(range(CORES)))
    node_logits = np.empty(nb, dtype=np.float32)
    partition_logits = np.empty((nb, PQ), dtype=np.float32)
    for c in range(CORES):
        gp = glob_pos[c]
        v = gp >= 0
        node_logits[gp[v]] = res.results[c]["ns_out"][v]
        partition_logits[gp[v]] = res.results[c]["ps_out"][v]
    return node_logits, partition_logits
